# revision 27
# baseline (speedup 1.0000x reference)
"""Multi-head attention (B=2, S=2048, E=1024, H=16) on 8 TRN2 NeuronCores.

Sharding: batch x head-group. Core c handles batch b=c//4 and head group
g=c%4 (4 heads = 256 of E). Each core computes its heads' attention output
slice and a partial fc_out product [S, E]; the host sums the 4 partials per
batch and adds b_out.

Device-side math per core (all matmuls in float32r, full PE rate):
  qpT = (Wq_g @ q[b].T + bq)      [256, S]   (T layout: dims on partitions)
  kpT = (Wk_g @ k_c[b].T + bk)    [256, SKV] (k compressed by mask, padded)
  vp  = (v_c[b] @ Wv_g.T + bv)*m  [SKV, 4*65] (per head: 64 dims + ones col)
  S_T = kpT_h.T-chunks @ qpT_h    [SKV, S] per head (2 heads row-packed, K=64)
  E_T = exp(S_T)                  (no max-subtraction: |energy| <~ 60, safe)
  AV  = vp_aug.T @ E_T  -> [65, S]: rows 0-63 = unnormalized O_T, row 64 =
        softmax denominator (ones-column trick; pad rows contribute 0)
  O_T = AV[0:64] / AV[64]         (recip + PE partition-broadcast)
  out_partial = O_T.T @ Wo_g.T    [S, E] (both head-pairs accumulated in PSUM)

Mask handling is exact: masked K/V rows are removed on the host (gather),
so softmax(where(mask==0, -1e20, e)) == exp(e_valid)/sum(exp(e_valid)).

Pipeline: the attention loop is software-pipelined over 512-wide query
blocks: block s emits [AV of block s-1 | scores of block s | exp of block s]
interleaved per skv-chunk, so the Scalar engine (exp, the per-block
bottleneck) always has scores available and the PE back-fills with AV,
Q-projection (pt=0) or fc_out (pt=1) work. exp runs as one 1024-wide
instruction per skv-chunk over both row-packed heads (2 adjacent PSUM
banks) to amortize ACT fixed overheads. Softmax normalization is per-block
so fc_out can start before the whole pair finishes.
"""

import os

import ml_dtypes
import numpy as np

B, S, E, H = 2, 2048, 1024, 16
D = E // H           # 64
NCORES = 8
GROUPS = 4           # head groups per batch (cores per batch)
HPG = H // GROUPS    # 4 heads per core
DC = E // GROUPS     # 256 dims per core
NB = E // 128        # 8 contraction chunks over E
QB = 512             # query block width
NQB = S // QB        # 4

_CACHE = {}


def _split_excess_waits(nc, max_waits=1):
    """walrus rejects instructions carrying >1 sem wait; spread extras onto
    single-wait NoOps inserted before the instruction on the same engine."""
    import concourse.mybir as mybir

    n_split = 0
    for f in nc.m.functions:
        for bb in f.blocks:
            out, changed = [], False
            for ins in bb.instructions:
                si = ins.sync_info
                if si is not None and si.on_wait is not None and len(si.on_wait) > max_waits:
                    waits = list(si.on_wait)
                    for w in waits[:-max_waits]:
                        out.append(mybir.InstNoOp(
                            name=nc.get_next_instruction_name(),
                            engine=ins.engine, ins=[], outs=[],
                            sync_info=mybir.SyncInfo(on_wait=[w], on_update=[])))
                        n_split += 1
                    ins.sync_info = mybir.SyncInfo(
                        on_wait=waits[-max_waits:], on_update=list(si.on_update))
                    changed = True
                out.append(ins)
            if changed:
                bb.instructions = out
    return n_split


def _build(skv, split_waits=True):
    import concourse.bass as bass
    import concourse.mybir as mybir
    import concourse.tile as tile

    f32 = mybir.dt.float32
    f32r = mybir.dt.float32r
    f16 = mybir.dt.float16
    bf16 = mybir.dt.bfloat16
    f8 = mybir.dt.float8e4
    Alu = mybir.AluOpType
    Act = mybir.ActivationFunctionType
    DR = mybir.MatmulPerfMode.DoubleRow

    nsk = skv // 128
    kblocks = []
    rem = skv
    while rem > 0:
        w = 384 if rem % 384 == 0 else min(256, rem)
        kblocks.append(w)
        rem -= w

    nc = bass.Bass()
    xqT = nc.declare_dram_parameter("xqT", [E, S], f16, isOutput=False)
    xkT = nc.declare_dram_parameter("xkT", [E, skv], f16, isOutput=False)
    xvT = nc.declare_dram_parameter("xvT", [E, skv], f16, isOutput=False)
    wqT = nc.declare_dram_parameter("wqT", [E, DC], f16, isOutput=False)
    wkT = nc.declare_dram_parameter("wkT", [E, DC], f16, isOutput=False)
    wvT = nc.declare_dram_parameter("wvT", [E, DC], f16, isOutput=False)
    woT = nc.declare_dram_parameter("woT", [DC, E], f16, isOutput=False)
    bq_d = nc.declare_dram_parameter("bq", [DC], f32, isOutput=False)
    bk_d = nc.declare_dram_parameter("bk", [DC], f32, isOutput=False)
    bv_d = nc.declare_dram_parameter("bv", [DC], f32, isOutput=False)
    vm_d = nc.declare_dram_parameter("vmask", [skv], f32, isOutput=False)
    sel2_d = nc.declare_dram_parameter("sel2", [2, 128], f32r, isOutput=False)
    out_d = nc.declare_dram_parameter("out", [S, E], f16, isOutput=True)

    xqT_r = xqT.rearrange("(ko p) s -> p ko s", p=128)
    xkT_r = xkT.rearrange("(ko p) s -> p ko s", p=128)
    xvT_r = xvT.rearrange("(ko p) s -> p ko s", p=128)

    with tile.TileContext(nc) as tc:
        with (
            tc.tile_pool(name="weights", bufs=1) as wpool,
            tc.tile_pool(name="consts", bufs=1) as cpool,
            tc.tile_pool(name="persist", bufs=1) as ppool,
            tc.tile_pool(name="small", bufs=2) as smpool,
            tc.tile_pool(name="stream", bufs=1) as spool,
            tc.tile_pool(name="px_ps", bufs=2, space="PSUM") as pxps,
            tc.tile_pool(name="av_ps", bufs=2, space="PSUM") as avps,
            tc.tile_pool(name="gen_ps", bufs=2, space="PSUM") as gps,
            tc.tile_pool(name="et", bufs=2) as etpool,
            tc.tile_pool(name="outp", bufs=3) as opool,
            tc.tile_pool(name="rcr", bufs=2) as rcpool,
        ):
            # ---- tiles ----
            wk_t = wpool.tile([128, NB, DC], f16, tag="wk", name="wk_t")
            wq_t = wpool.tile([128, NB, DC], f16, tag="wq", name="wq_t")
            wv_t = wpool.tile([128, NB, DC], f16, tag="wv", name="wv_t")
            wo_t = wpool.tile([128, DC // 128, E], f16, tag="wo", name="wo_t")
            bq_t = cpool.tile([128, 2], f32, tag="bq")
            bk_t = cpool.tile([128, 2], f32, tag="bk")
            bv_t = cpool.tile([128, DC], f32, tag="bv")
            vm_t = cpool.tile([128, nsk], f32, tag="vm")
            sel2_t = cpool.tile([2, 128], f32r, tag="sel2")
            xks = [spool.tile([128, NB, w], f16, tag=f"xk{i}", name="xk")
                   for i, w in enumerate(kblocks)]
            xqs = [spool.tile([128, NB, 512], f16, tag=f"xq{i}", name="xq")
                   for i in range(NQB)]
            xvs = [spool.tile([128, NB, 128], f16, tag=f"xv{i}", name="xv")
                  for i in range(nsk)]

            qpT = ppool.tile([128, 2, S], f16, tag="qpT")
            kpT = ppool.tile([128, 2, skv], f16, tag="kpT")
            vp = ppool.tile([128, nsk, HPG * (D + 1)], bf16, tag="vp")
            # per-j unnormalized O_T (rows 0-63) + denominator (row 64)
            o_un0 = ppool.tile([65, 2, S], f32, tag="o_un0")
            o_un1 = ppool.tile([65, 2, S], f32, tag="o_un1")
            o_uns = [o_un0, o_un1]
            o_f16 = ppool.tile([128, 2, S], f16, tag="o_f16")

            # ---- all input DMAs issued upfront in priority order; the
            # late-needed ones go on the scalar HWDGE queue (idle pre-exp) ----
            nc.sync.dma_start(sel2_t[:], sel2_d[:])
            nc.sync.dma_start(wk_t[:], wkT.rearrange("(ko p) m -> p ko m", p=128))
            nc.sync.dma_start(bk_t[:], bk_d.rearrange("(c p) -> p c", p=128))
            nc.sync.dma_start(bq_t[:], bq_d.rearrange("(c p) -> p c", p=128))
            nc.sync.dma_start(bv_t[:], bv_d[None, :].to_broadcast((128, DC)))
            nc.sync.dma_start(vm_t[:], vm_d.rearrange("(s p) -> p s", p=128))
            nc.sync.dma_start(xks[0][:], xkT_r[:, :, :kblocks[0]])
            nc.sync.dma_start(wq_t[:], wqT.rearrange("(ko p) m -> p ko m", p=128))
            nc.sync.dma_start(xqs[0][:], xqT_r[:, :, 0:512])
            off = kblocks[0]
            for i, w in enumerate(kblocks[1:], start=1):
                nc.sync.dma_start(xks[i][:], xkT_r[:, :, off:off + w])
                off += w
            nc.sync.dma_start(wv_t[:], wvT.rearrange("(ko p) m -> p ko m", p=128))
            nc.sync.dma_start(xqs[1][:], xqT_r[:, :, 512:1024])
            for sc in range(nsk):
                nc.scalar.dma_start(xvs[sc][:], xvT_r[:, :, sc * 128:(sc + 1) * 128])
            for nb in range(2, NQB):
                nc.scalar.dma_start(xqs[nb][:], xqT_r[:, :, nb * 512:(nb + 1) * 512])
            nc.scalar.dma_start(wo_t[:], woT.rearrange("(ko p) n -> p ko n", p=128))


            def mm_k():
                off = 0
                for i, w in enumerate(kblocks):
                    for mc in range(2):
                        ps = gps.tile([128, 512], f32, tag="gp", name="kp_ps")[:, :w]
                        for kc in range(NB):
                            nc.tensor.matmul(
                                ps[:], wk_t[:, kc, mc * 128:(mc + 1) * 128],
                                xks[i][:, kc, :], start=(kc == 0), stop=(kc == NB - 1))
                        nc.vector.tensor_tensor(
                            out=kpT[:, mc, off:off + w], in0=ps[:],
                            in1=bk_t[:, mc:mc + 1].to_broadcast((128, w)), op=Alu.add)
                    off += w

            def mm_q(nb):
                for mc in range(2):
                    ps = gps.tile([128, 512], f32, tag="gp", name="qp_ps")
                    for kc in range(NB):
                        nc.tensor.matmul(
                            ps[:], wq_t[:, kc, mc * 128:(mc + 1) * 128],
                            xqs[nb][:, kc, :], start=(kc == 0), stop=(kc == NB - 1))
                    nc.vector.tensor_tensor(
                        out=qpT[:, mc, nb * 512:(nb + 1) * 512], in0=ps[:],
                        in1=bq_t[:, mc:mc + 1].to_broadcast((128, 512)), op=Alu.add)

            def mm_v(sc):
                ps = gps.tile([128, 512], f32, tag="gp", name="vp_ps")[:, :DC]
                for kc in range(NB):
                    nc.tensor.matmul(
                        ps[:], xvs[sc][:, kc, :], wv_t[:, kc, :],
                        start=(kc == 0), stop=(kc == NB - 1))
                t1 = smpool.tile([128, DC], f32, tag="vtmp")
                nc.vector.tensor_tensor(out=t1[:], in0=ps[:], in1=bv_t[:], op=Alu.add)
                vps = vp[:, sc, :].rearrange("p (h w) -> p h w", w=D + 1)
                nc.vector.tensor_tensor(
                    out=vps[:, :, 0:D],
                    in0=t1.rearrange("p (h w) -> p h w", w=D),
                    in1=vm_t[:, sc:sc + 1, None].to_broadcast((128, HPG, D)),
                    op=Alu.mult)
                nc.vector.tensor_copy(
                    out=vps[:, :, D:D + 1],
                    in_=vm_t[:, sc:sc + 1, None].to_broadcast((128, HPG, 1)))

            def finish_block(pt, qb, psavs):
                """AV copy + per-block softmax normalize for query block qb
                of pair pt (runs one pipeline step after its AV matmuls)."""
                q0 = qb * QB
                for j in range(2):
                    nc.vector.tensor_copy(
                        out=o_uns[j][0:D + 1, pt, q0:q0 + QB],
                        in_=psavs[j][0:D + 1, :])
                rc_r = rcpool.tile([2, QB], f32r, tag="rcr")
                for j in range(2):
                    s128 = smpool.tile([128, QB // 128], f32, tag="s128")
                    nc.gpsimd.dma_start(s128[:], o_uns[j][D:D + 1, pt, q0:q0 + QB])
                    nc.vector.reciprocal(out=s128[:], in_=s128[:])
                    r128 = smpool.tile([128, QB // 128], f32r, tag="r128")
                    nc.vector.tensor_copy(out=r128[:], in_=s128[:])
                    nc.gpsimd.dma_start(rc_r[j:j + 1, :], r128[:])
                # one PE broadcast for both heads: sel2 routes row j of rc_r
                # to partitions 64j..64j+63
                rc_ps = gps.tile([128, 512], f32, tag="gp", name="rc_ps")
                nc.tensor.matmul(
                    rc_ps[:], sel2_t[:], rc_r[:], start=True, stop=True)
                for j in range(2):
                    nc.vector.tensor_tensor(
                        out=o_f16[64 * j:64 * j + 64, pt, q0:q0 + QB],
                        in0=o_uns[j][0:D, pt, q0:q0 + QB],
                        in1=rc_ps[64 * j:64 * j + 64, :], op=Alu.mult)

            def fc_block(b):
                """fc_out for query block b (both pairs accumulated in PSUM)."""
                for sqc in range(b * (QB // 128), (b + 1) * (QB // 128)):
                    ob = opool.tile([128, 2, 512], f16, tag="ob")
                    for eb in range(2):
                        ps = gps.tile([128, 512], f32, tag="gp", name="fc_ps")
                        nc.tensor.matmul(
                            ps[:], o_f16[:, 0, sqc * 128:(sqc + 1) * 128],
                            wo_t[:, 0, eb * 512:(eb + 1) * 512],
                            start=True, stop=False)
                        nc.tensor.matmul(
                            ps[:], o_f16[:, 1, sqc * 128:(sqc + 1) * 128],
                            wo_t[:, 1, eb * 512:(eb + 1) * 512],
                            start=False, stop=True)
                        nc.vector.tensor_copy(out=ob[:, eb, :], in_=ps[:])
                    nc.sync.dma_start(
                        out_d[sqc * 128:(sqc + 1) * 128, :], ob[:])

            # lead-in: kpT fully, first qpT block (vp projected inside step 0)
            mm_k()
            mm_q(0)
            # HAM warm-up: junk matmuls at idle-filler priority keep the PE
            # activity window busy while the input streams arrive, so the
            # real projection matmuls run at full clock.
            with tc.high_priority(offset=-10**6):
                for _ in range(12):
                    wps = gps.tile([128, 512], f32, tag="gp", name="warm_ps")
                    nc.tensor.matmul(wps[:, :128], sel2_t[:], sel2_t[:],
                                     start=True, stop=True)

            def av_chunk(state, psavs, sk0, sk1):
                """AV matmuls for skc in [sk0, sk1) of the previous block,
                one contiguous accumulation chain per head."""
                ppt, pqb, pet = state
                for j in range(2):
                    hl = 2 * ppt + j
                    for skc in range(sk0, sk1):
                        nc.tensor.matmul(
                            psavs[j][0:D + 1, :],
                            vp[:, skc, hl * (D + 1):(hl + 1) * (D + 1)],
                            pet[:, skc, j, :],
                            start=(skc == 0), stop=(skc == nsk - 1))

            # software-pipelined attention over 8 (pt, qb) steps
            state = None  # (pt, qb, et tile) awaiting AV
            GRP = 4
            for pt in range(2):
                for qb in range(NQB):
                    q0 = qb * QB
                    et_t = etpool.tile([128, nsk, 2, QB], bf16, tag="et",
                                       name="et_t")
                    psavs = None
                    if state is not None:
                        psavs = [avps.tile([128, QB], f32, tag="av",
                                           name=f"psav{j}") for j in range(2)]
                    for g0 in range(0, nsk, GRP):
                        g1 = min(g0 + GRP, nsk)
                        if state is not None:
                            av_chunk(state, psavs, g0, g1)
                        for skc in range(g0, g1):
                            psx = pxps.tile([128, 2, QB], f32, tag="px",
                                            name="psx")
                            for j in range(2):
                                nc.tensor.matmul(
                                    psx[:, j, :],
                                    kpT[64 * j:64 * j + 64, pt,
                                        skc * 128:(skc + 1) * 128],
                                    qpT[64 * j:64 * j + 64, pt, q0:q0 + QB],
                                    start=True, stop=True,
                                    tile_position=(64 * j, 0))
                            nc.scalar.activation(
                                et_t[:, skc, :, :], psx[:], Act.Exp)
                    # PE filler during the ACT-bound exp phase. In pt=0 the
                    # filler is emitted before finish_block so its matmuls
                    # outrank the normalize chain; in pt=1 fc needs the
                    # normalize, so finish goes first.
                    if pt == 0:
                        if qb + 1 < NQB:
                            mm_q(qb + 1)
                        if qb == 0:
                            # vp must be complete before block-0's AV, which
                            # is emitted at the start of step 1
                            for sc in range(nsk):
                                mm_v(sc)
                        if state is not None:
                            finish_block(state[0], state[1], psavs)
                    else:
                        if state is not None:
                            finish_block(state[0], state[1], psavs)
                        if qb >= 1:
                            fc_block(qb - 1)
                    state = (pt, qb, et_t)
            # drain: AV + normalize of the last block, then remaining fc
            psavs = [avps.tile([128, QB], f32, tag="av", name=f"psavd{j}")
                     for j in range(2)]
            av_chunk(state, psavs, 0, nsk)
            finish_block(state[0], state[1], psavs)
            fc_block(3)

    if split_waits:
        _split_excess_waits(nc)
    return nc


def _prep_inputs(q, k, v, mask, W_qkv, b_qkv, W_out, b_out):
    """Host-side shard/layout prep. Returns (skv, in_maps)."""
    q = np.asarray(q, dtype=np.float32)
    k = np.asarray(k, dtype=np.float32)
    v = np.asarray(v, dtype=np.float32)
    mask = np.asarray(mask)
    W_qkv = np.asarray(W_qkv, dtype=np.float32)
    b_qkv = np.asarray(b_qkv, dtype=np.float32)
    W_out = np.asarray(W_out, dtype=np.float32)

    valid = [np.nonzero(mask[b, 0, 0] != 0)[0] for b in range(B)]
    cnts = [len(vi) for vi in valid]
    skv = max(128, max((c + 127) // 128 * 128 for c in cnts))

    # per-batch tensors
    qT, kTc, vTc, vms = [], [], [], []
    for b in range(B):
        qT.append(np.ascontiguousarray(q[b].T).astype(np.float16))
        kt = np.zeros((E, skv), np.float16)
        vt = np.zeros((E, skv), np.float16)
        kt[:, :cnts[b]] = k[b][valid[b]].T
        vt[:, :cnts[b]] = v[b][valid[b]].T
        kTc.append(kt)
        vTc.append(vt)
        vm = np.zeros((skv,), np.float32)
        vm[:cnts[b]] = 1.0
        vms.append(vm)

    sel2 = np.zeros((2, 128), np.float32)
    sel2[0, :64] = 1.0
    sel2[1, 64:] = 1.0

    in_maps = []
    for c in range(NCORES):
        b, g = divmod(c, GROUPS)
        sl = slice(g * DC, (g + 1) * DC)
        in_maps.append({
            "xqT": qT[b], "xkT": kTc[b], "xvT": vTc[b],
            "wqT": np.ascontiguousarray(W_qkv[sl, :].T).astype(np.float16),
            "wkT": np.ascontiguousarray(W_qkv[E:][sl, :].T).astype(np.float16),
            "wvT": np.ascontiguousarray(W_qkv[2 * E:][sl, :].T).astype(np.float16),
            "woT": np.ascontiguousarray(W_out[:, sl].T).astype(np.float16),
            "bq": np.ascontiguousarray(b_qkv[sl]),
            "bk": np.ascontiguousarray(b_qkv[E:][sl]),
            "bv": np.ascontiguousarray(b_qkv[2 * E:][sl]),
            "vmask": vms[b],
            "sel2": sel2,
        })
    return skv, in_maps


def kernel(q, k, v, mask, W_qkv, b_qkv, W_out, b_out):
    from concourse import bass_utils

    skv, in_maps = _prep_inputs(q, k, v, mask, W_qkv, b_qkv, W_out, b_out)
    if skv not in _CACHE:
        _CACHE[skv] = _build(skv)
    nc = _CACHE[skv]

    trace = os.environ.get("KERNEL_TRACE") == "1"
    if trace:
        bass_utils.upload_artifacts = lambda tmpdir: "local://" + tmpdir
    res = bass_utils.run_bass_kernel_spmd(
        nc, in_maps, list(range(NCORES)), trace=trace)
    if trace:
        print(f"HW exec time: {res.exec_time_ns} ns")

    b_out = np.asarray(b_out, dtype=np.float32)
    out = np.zeros((B, S, E), np.float32)
    for c in range(NCORES):
        out[c // GROUPS] += res.results[c]["out"].astype(np.float32)
    out += b_out[None, None, :]
    return out


# revision 28
# speedup vs baseline: 1.0678x; 1.0678x over previous
"""Multi-head attention (B=2, S=2048, E=1024, H=16) on 8 TRN2 NeuronCores.

Sharding: batch x head-group. Core c handles batch b=c//4 and head group
g=c%4 (4 heads = 256 of E). Each core computes its heads' attention output
slice and a partial fc_out product [S, E]; the host sums the 4 partials per
batch and adds b_out.

Device-side math per core (all matmuls in float32r, full PE rate):
  qpT = (Wq_g @ q[b].T + bq)      [256, S]   (T layout: dims on partitions)
  kpT = (Wk_g @ k_c[b].T + bk)    [256, SKV] (k compressed by mask, padded)
  vp  = (v_c[b] @ Wv_g.T + bv)*m  [SKV, 4*65] (per head: 64 dims + ones col)
  S_T = kpT_h.T-chunks @ qpT_h    [SKV, S] per head (2 heads row-packed, K=64)
  E_T = exp(S_T)                  (no max-subtraction: |energy| <~ 60, safe)
  AV  = vp_aug.T @ E_T  -> [65, S]: rows 0-63 = unnormalized O_T, row 64 =
        softmax denominator (ones-column trick; pad rows contribute 0)
  O_T = AV[0:64] / AV[64]         (recip + PE partition-broadcast)
  out_partial = O_T.T @ Wo_g.T    [S, E] (both head-pairs accumulated in PSUM)

Mask handling is exact: masked K/V rows are removed on the host (gather),
so softmax(where(mask==0, -1e20, e)) == exp(e_valid)/sum(exp(e_valid)).

Pipeline: the attention loop is software-pipelined over 512-wide query
blocks: block s emits [AV of block s-1 | scores of block s | exp of block s]
interleaved per skv-chunk, so the Scalar engine (exp, the per-block
bottleneck) always has scores available and the PE back-fills with AV,
Q-projection (pt=0) or fc_out (pt=1) work. exp runs as one 1024-wide
instruction per skv-chunk over both row-packed heads (2 adjacent PSUM
banks) to amortize ACT fixed overheads. Softmax normalization is per-block
so fc_out can start before the whole pair finishes.
"""

import os

import ml_dtypes
import numpy as np

B, S, E, H = 2, 2048, 1024, 16
D = E // H           # 64
NCORES = 8
GROUPS = 4           # head groups per batch (cores per batch)
HPG = H // GROUPS    # 4 heads per core
DC = E // GROUPS     # 256 dims per core
NB = E // 128        # 8 contraction chunks over E
QB = 512             # query block width
NQB = S // QB        # 4

_CACHE = {}


def _split_excess_waits(nc, max_waits=1):
    """walrus rejects instructions carrying >1 sem wait; spread extras onto
    single-wait NoOps inserted before the instruction on the same engine."""
    import concourse.mybir as mybir

    n_split = 0
    for f in nc.m.functions:
        for bb in f.blocks:
            out, changed = [], False
            for ins in bb.instructions:
                si = ins.sync_info
                if si is not None and si.on_wait is not None and len(si.on_wait) > max_waits:
                    waits = list(si.on_wait)
                    for w in waits[:-max_waits]:
                        out.append(mybir.InstNoOp(
                            name=nc.get_next_instruction_name(),
                            engine=ins.engine, ins=[], outs=[],
                            sync_info=mybir.SyncInfo(on_wait=[w], on_update=[])))
                        n_split += 1
                    ins.sync_info = mybir.SyncInfo(
                        on_wait=waits[-max_waits:], on_update=list(si.on_update))
                    changed = True
                out.append(ins)
            if changed:
                bb.instructions = out
    return n_split


def _build(skv, split_waits=True):
    import concourse.bass as bass
    import concourse.mybir as mybir
    import concourse.tile as tile

    f32 = mybir.dt.float32
    f32r = mybir.dt.float32r
    f16 = mybir.dt.float16
    bf16 = mybir.dt.bfloat16
    f8 = mybir.dt.float8e4
    Alu = mybir.AluOpType
    Act = mybir.ActivationFunctionType
    DR = mybir.MatmulPerfMode.DoubleRow

    nsk = skv // 128
    kblocks = []
    rem = skv
    while rem > 0:
        w = 384 if rem % 384 == 0 else min(256, rem)
        kblocks.append(w)
        rem -= w

    nc = bass.Bass()
    xqT = nc.declare_dram_parameter("xqT", [E, S], f16, isOutput=False)
    xkT = nc.declare_dram_parameter("xkT", [E, skv], f16, isOutput=False)
    xvT = nc.declare_dram_parameter("xvT", [E, skv], f16, isOutput=False)
    wqT = nc.declare_dram_parameter("wqT", [E, DC], f16, isOutput=False)
    wkT = nc.declare_dram_parameter("wkT", [E, DC], f16, isOutput=False)
    wvT = nc.declare_dram_parameter("wvT", [E, DC], f16, isOutput=False)
    woT = nc.declare_dram_parameter("woT", [DC, E], f16, isOutput=False)
    bq_d = nc.declare_dram_parameter("bq", [DC], f32, isOutput=False)
    bk_d = nc.declare_dram_parameter("bk", [DC], f32, isOutput=False)
    bv_d = nc.declare_dram_parameter("bv", [DC], f32, isOutput=False)
    vm_d = nc.declare_dram_parameter("vmask", [skv], f32, isOutput=False)
    sel2_d = nc.declare_dram_parameter("sel2", [2, 128], f32r, isOutput=False)
    out_d = nc.declare_dram_parameter("out", [S, E], f16, isOutput=True)

    xqT_r = xqT.rearrange("(ko p) s -> p ko s", p=128)
    xkT_r = xkT.rearrange("(ko p) s -> p ko s", p=128)
    xvT_r = xvT.rearrange("(ko p) s -> p ko s", p=128)

    with tile.TileContext(nc) as tc:
        with (
            tc.tile_pool(name="weights", bufs=1) as wpool,
            tc.tile_pool(name="consts", bufs=1) as cpool,
            tc.tile_pool(name="persist", bufs=1) as ppool,
            tc.tile_pool(name="small", bufs=2) as smpool,
            tc.tile_pool(name="stream", bufs=1) as spool,
            tc.tile_pool(name="px_ps", bufs=2, space="PSUM") as pxps,
            tc.tile_pool(name="av_ps", bufs=2, space="PSUM") as avps,
            tc.tile_pool(name="gen_ps", bufs=2, space="PSUM") as gps,
            tc.tile_pool(name="et", bufs=2) as etpool,
            tc.tile_pool(name="outp", bufs=3) as opool,
            tc.tile_pool(name="rcr", bufs=2) as rcpool,
        ):
            # ---- tiles ----
            wk_t = wpool.tile([128, NB, DC], f16, tag="wk", name="wk_t")
            wq_t = wpool.tile([128, NB, DC], f16, tag="wq", name="wq_t")
            wv_t = wpool.tile([128, NB, DC], f16, tag="wv", name="wv_t")
            wo_t = wpool.tile([128, DC // 128, E], f16, tag="wo", name="wo_t")
            bq_t = cpool.tile([128, 2], f32, tag="bq")
            bk_t = cpool.tile([128, 2], f32, tag="bk")
            bv_t = cpool.tile([128, DC], f32, tag="bv")
            vm_t = cpool.tile([128, nsk], f32, tag="vm")
            sel2_t = cpool.tile([2, 128], f32r, tag="sel2")
            xks = [spool.tile([128, NB, w], f16, tag=f"xk{i}", name="xk")
                   for i, w in enumerate(kblocks)]
            xqs = [spool.tile([128, NB, 512], f16, tag=f"xq{i}", name="xq")
                   for i in range(NQB)]
            xvs = [spool.tile([128, NB, 128], f16, tag=f"xv{i}", name="xv")
                  for i in range(nsk)]

            qpT = ppool.tile([128, 2, S], f16, tag="qpT")
            kpT = ppool.tile([128, 2, skv], f16, tag="kpT")
            vp = ppool.tile([128, nsk, HPG * (D + 1)], bf16, tag="vp")
            # per-j unnormalized O_T (rows 0-63) + denominator (row 64)
            o_un0 = ppool.tile([65, 2, S], f32, tag="o_un0")
            o_un1 = ppool.tile([65, 2, S], f32, tag="o_un1")
            o_uns = [o_un0, o_un1]
            o_f16 = ppool.tile([128, 2, S], f16, tag="o_f16")

            # ---- all input DMAs issued upfront in priority order; the
            # late-needed ones go on the scalar HWDGE queue (idle pre-exp) ----
            nc.sync.dma_start(sel2_t[:], sel2_d[:])
            nc.sync.dma_start(wk_t[:], wkT.rearrange("(ko p) m -> p ko m", p=128))
            nc.sync.dma_start(bk_t[:], bk_d.rearrange("(c p) -> p c", p=128))
            nc.sync.dma_start(bq_t[:], bq_d.rearrange("(c p) -> p c", p=128))
            nc.sync.dma_start(bv_t[:], bv_d[None, :].to_broadcast((128, DC)))
            nc.sync.dma_start(vm_t[:], vm_d.rearrange("(s p) -> p s", p=128))
            nc.sync.dma_start(xks[0][:], xkT_r[:, :, :kblocks[0]])
            nc.sync.dma_start(wq_t[:], wqT.rearrange("(ko p) m -> p ko m", p=128))
            nc.sync.dma_start(xqs[0][:], xqT_r[:, :, 0:512])
            off = kblocks[0]
            for i, w in enumerate(kblocks[1:], start=1):
                nc.sync.dma_start(xks[i][:], xkT_r[:, :, off:off + w])
                off += w
            nc.sync.dma_start(wv_t[:], wvT.rearrange("(ko p) m -> p ko m", p=128))
            nc.sync.dma_start(xqs[1][:], xqT_r[:, :, 512:1024])
            for sc in range(nsk):
                nc.sync.dma_start(xvs[sc][:], xvT_r[:, :, sc * 128:(sc + 1) * 128])
            for nb in range(2, NQB):
                nc.sync.dma_start(xqs[nb][:], xqT_r[:, :, nb * 512:(nb + 1) * 512])
            nc.sync.dma_start(wo_t[:], woT.rearrange("(ko p) n -> p ko n", p=128))


            def mm_k():
                off = 0
                for i, w in enumerate(kblocks):
                    for mc in range(2):
                        ps = gps.tile([128, 512], f32, tag="gp", name="kp_ps")[:, :w]
                        for kc in range(NB):
                            nc.tensor.matmul(
                                ps[:], wk_t[:, kc, mc * 128:(mc + 1) * 128],
                                xks[i][:, kc, :], start=(kc == 0), stop=(kc == NB - 1))
                        nc.vector.tensor_tensor(
                            out=kpT[:, mc, off:off + w], in0=ps[:],
                            in1=bk_t[:, mc:mc + 1].to_broadcast((128, w)), op=Alu.add)
                    off += w

            def mm_q(nb):
                for mc in range(2):
                    ps = gps.tile([128, 512], f32, tag="gp", name="qp_ps")
                    for kc in range(NB):
                        nc.tensor.matmul(
                            ps[:], wq_t[:, kc, mc * 128:(mc + 1) * 128],
                            xqs[nb][:, kc, :], start=(kc == 0), stop=(kc == NB - 1))
                    nc.vector.tensor_tensor(
                        out=qpT[:, mc, nb * 512:(nb + 1) * 512], in0=ps[:],
                        in1=bq_t[:, mc:mc + 1].to_broadcast((128, 512)), op=Alu.add)

            def mm_v(sc):
                ps = gps.tile([128, 512], f32, tag="gp", name="vp_ps")[:, :DC]
                for kc in range(NB):
                    nc.tensor.matmul(
                        ps[:], xvs[sc][:, kc, :], wv_t[:, kc, :],
                        start=(kc == 0), stop=(kc == NB - 1))
                t1 = smpool.tile([128, DC], f32, tag="vtmp")
                nc.vector.tensor_tensor(out=t1[:], in0=ps[:], in1=bv_t[:], op=Alu.add)
                vps = vp[:, sc, :].rearrange("p (h w) -> p h w", w=D + 1)
                nc.vector.tensor_tensor(
                    out=vps[:, :, 0:D],
                    in0=t1.rearrange("p (h w) -> p h w", w=D),
                    in1=vm_t[:, sc:sc + 1, None].to_broadcast((128, HPG, D)),
                    op=Alu.mult)
                nc.vector.tensor_copy(
                    out=vps[:, :, D:D + 1],
                    in_=vm_t[:, sc:sc + 1, None].to_broadcast((128, HPG, 1)))

            def finish_block(pt, qb, psavs):
                """AV copy + per-block softmax normalize for query block qb
                of pair pt (runs one pipeline step after its AV matmuls)."""
                q0 = qb * QB
                for j in range(2):
                    nc.vector.tensor_copy(
                        out=o_uns[j][0:D + 1, pt, q0:q0 + QB],
                        in_=psavs[j][0:D + 1, :])
                rc_r = rcpool.tile([2, QB], f32r, tag="rcr")
                for j in range(2):
                    s128 = smpool.tile([128, QB // 128], f32, tag="s128")
                    nc.gpsimd.dma_start(s128[:], o_uns[j][D:D + 1, pt, q0:q0 + QB])
                    nc.vector.reciprocal(out=s128[:], in_=s128[:])
                    r128 = smpool.tile([128, QB // 128], f32r, tag="r128")
                    nc.vector.tensor_copy(out=r128[:], in_=s128[:])
                    nc.gpsimd.dma_start(rc_r[j:j + 1, :], r128[:])
                # one PE broadcast for both heads: sel2 routes row j of rc_r
                # to partitions 64j..64j+63
                rc_ps = gps.tile([128, 512], f32, tag="gp", name="rc_ps")
                nc.tensor.matmul(
                    rc_ps[:], sel2_t[:], rc_r[:], start=True, stop=True)
                for j in range(2):
                    nc.vector.tensor_tensor(
                        out=o_f16[64 * j:64 * j + 64, pt, q0:q0 + QB],
                        in0=o_uns[j][0:D, pt, q0:q0 + QB],
                        in1=rc_ps[64 * j:64 * j + 64, :], op=Alu.mult)

            def fc_block(b):
                """fc_out for query block b (both pairs accumulated in PSUM)."""
                for sqc in range(b * (QB // 128), (b + 1) * (QB // 128)):
                    ob = opool.tile([128, 2, 512], f16, tag="ob")
                    for eb in range(2):
                        ps = gps.tile([128, 512], f32, tag="gp", name="fc_ps")
                        nc.tensor.matmul(
                            ps[:], o_f16[:, 0, sqc * 128:(sqc + 1) * 128],
                            wo_t[:, 0, eb * 512:(eb + 1) * 512],
                            start=True, stop=False)
                        nc.tensor.matmul(
                            ps[:], o_f16[:, 1, sqc * 128:(sqc + 1) * 128],
                            wo_t[:, 1, eb * 512:(eb + 1) * 512],
                            start=False, stop=True)
                        nc.vector.tensor_copy(out=ob[:, eb, :], in_=ps[:])
                    nc.sync.dma_start(
                        out_d[sqc * 128:(sqc + 1) * 128, :], ob[:])

            # HAM warm-up: junk matmuls at idle-filler priority keep the PE
            # activity window busy while the input streams arrive, so the
            # real projection matmuls run at full clock. Emitted before mm_k
            # so their PSUM ring slots precede the projections'.
            with tc.high_priority(offset=-10**6):
                for _ in range(12):
                    wps = gps.tile([128, 512], f32, tag="gp", name="warm_ps")
                    nc.tensor.matmul(wps[:, :128], sel2_t[:], sel2_t[:],
                                     start=True, stop=True)
            # lead-in: kpT fully, first qpT block (vp projected inside step 0)
            mm_k()
            mm_q(0)

            def av_chunk(state, psavs, sk0, sk1):
                """AV matmuls for skc in [sk0, sk1) of the previous block,
                one contiguous accumulation chain per head."""
                ppt, pqb, pet = state
                for j in range(2):
                    hl = 2 * ppt + j
                    for skc in range(sk0, sk1):
                        nc.tensor.matmul(
                            psavs[j][0:D + 1, :],
                            vp[:, skc, hl * (D + 1):(hl + 1) * (D + 1)],
                            pet[:, skc, j, :],
                            start=(skc == 0), stop=(skc == nsk - 1))

            # software-pipelined attention over 8 (pt, qb) steps
            state = None  # (pt, qb, et tile) awaiting AV
            GRP = 4
            for pt in range(2):
                for qb in range(NQB):
                    q0 = qb * QB
                    et_t = etpool.tile([128, nsk, 2, QB], bf16, tag="et",
                                       name="et_t")
                    psavs = None
                    if state is not None:
                        psavs = [avps.tile([128, QB], f32, tag="av",
                                           name=f"psav{j}") for j in range(2)]
                    for g0 in range(0, nsk, GRP):
                        g1 = min(g0 + GRP, nsk)
                        if state is not None:
                            av_chunk(state, psavs, g0, g1)
                        for skc in range(g0, g1):
                            psx = pxps.tile([128, 2, QB], f32, tag="px",
                                            name="psx")
                            for j in range(2):
                                nc.tensor.matmul(
                                    psx[:, j, :],
                                    kpT[64 * j:64 * j + 64, pt,
                                        skc * 128:(skc + 1) * 128],
                                    qpT[64 * j:64 * j + 64, pt, q0:q0 + QB],
                                    start=True, stop=True,
                                    tile_position=(64 * j, 0))
                            nc.scalar.activation(
                                et_t[:, skc, :, :], psx[:], Act.Exp)
                    # PE filler during the ACT-bound exp phase. In pt=0 the
                    # filler is emitted before finish_block so its matmuls
                    # outrank the normalize chain; in pt=1 fc needs the
                    # normalize, so finish goes first.
                    if pt == 0:
                        if qb + 1 < NQB:
                            mm_q(qb + 1)
                        if qb == 0:
                            # vp must be complete before block-0's AV, which
                            # is emitted at the start of step 1
                            for sc in range(nsk):
                                mm_v(sc)
                        if state is not None:
                            finish_block(state[0], state[1], psavs)
                    else:
                        if state is not None:
                            finish_block(state[0], state[1], psavs)
                        if qb >= 1:
                            fc_block(qb - 1)
                    state = (pt, qb, et_t)
            # drain: AV + normalize of the last block, then remaining fc
            psavs = [avps.tile([128, QB], f32, tag="av", name=f"psavd{j}")
                     for j in range(2)]
            av_chunk(state, psavs, 0, nsk)
            finish_block(state[0], state[1], psavs)
            fc_block(3)

    if split_waits:
        _split_excess_waits(nc)
    return nc


def _prep_inputs(q, k, v, mask, W_qkv, b_qkv, W_out, b_out):
    """Host-side shard/layout prep. Returns (skv, in_maps)."""
    q = np.asarray(q, dtype=np.float32)
    k = np.asarray(k, dtype=np.float32)
    v = np.asarray(v, dtype=np.float32)
    mask = np.asarray(mask)
    W_qkv = np.asarray(W_qkv, dtype=np.float32)
    b_qkv = np.asarray(b_qkv, dtype=np.float32)
    W_out = np.asarray(W_out, dtype=np.float32)

    valid = [np.nonzero(mask[b, 0, 0] != 0)[0] for b in range(B)]
    cnts = [len(vi) for vi in valid]
    skv = max(128, max((c + 127) // 128 * 128 for c in cnts))

    # per-batch tensors
    qT, kTc, vTc, vms = [], [], [], []
    for b in range(B):
        qT.append(np.ascontiguousarray(q[b].T).astype(np.float16))
        kt = np.zeros((E, skv), np.float16)
        vt = np.zeros((E, skv), np.float16)
        kt[:, :cnts[b]] = k[b][valid[b]].T
        vt[:, :cnts[b]] = v[b][valid[b]].T
        kTc.append(kt)
        vTc.append(vt)
        vm = np.zeros((skv,), np.float32)
        vm[:cnts[b]] = 1.0
        vms.append(vm)

    sel2 = np.zeros((2, 128), np.float32)
    sel2[0, :64] = 1.0
    sel2[1, 64:] = 1.0

    in_maps = []
    for c in range(NCORES):
        b, g = divmod(c, GROUPS)
        sl = slice(g * DC, (g + 1) * DC)
        in_maps.append({
            "xqT": qT[b], "xkT": kTc[b], "xvT": vTc[b],
            "wqT": np.ascontiguousarray(W_qkv[sl, :].T).astype(np.float16),
            "wkT": np.ascontiguousarray(W_qkv[E:][sl, :].T).astype(np.float16),
            "wvT": np.ascontiguousarray(W_qkv[2 * E:][sl, :].T).astype(np.float16),
            "woT": np.ascontiguousarray(W_out[:, sl].T).astype(np.float16),
            "bq": np.ascontiguousarray(b_qkv[sl]),
            "bk": np.ascontiguousarray(b_qkv[E:][sl]),
            "bv": np.ascontiguousarray(b_qkv[2 * E:][sl]),
            "vmask": vms[b],
            "sel2": sel2,
        })
    return skv, in_maps


def kernel(q, k, v, mask, W_qkv, b_qkv, W_out, b_out):
    from concourse import bass_utils

    skv, in_maps = _prep_inputs(q, k, v, mask, W_qkv, b_qkv, W_out, b_out)
    if skv not in _CACHE:
        _CACHE[skv] = _build(skv)
    nc = _CACHE[skv]

    trace = os.environ.get("KERNEL_TRACE") == "1"
    if trace:
        bass_utils.upload_artifacts = lambda tmpdir: "local://" + tmpdir
    res = bass_utils.run_bass_kernel_spmd(
        nc, in_maps, list(range(NCORES)), trace=trace)
    if trace:
        print(f"HW exec time: {res.exec_time_ns} ns")

    b_out = np.asarray(b_out, dtype=np.float32)
    out = np.zeros((B, S, E), np.float32)
    for c in range(NCORES):
        out[c // GROUPS] += res.results[c]["out"].astype(np.float32)
    out += b_out[None, None, :]
    return out


# revision 30
# speedup vs baseline: 1.1014x; 1.0315x over previous
"""Multi-head attention (B=2, S=2048, E=1024, H=16) on 8 TRN2 NeuronCores.

Sharding: batch x head-group. Core c handles batch b=c//4 and head group
g=c%4 (4 heads = 256 of E). Each core computes its heads' attention output
slice and a partial fc_out product [S, E]; the host sums the 4 partials per
batch and adds b_out.

Device-side math per core (all matmuls in float32r, full PE rate):
  qpT = (Wq_g @ q[b].T + bq)      [256, S]   (T layout: dims on partitions)
  kpT = (Wk_g @ k_c[b].T + bk)    [256, SKV] (k compressed by mask, padded)
  vp  = (v_c[b] @ Wv_g.T + bv)*m  [SKV, 4*65] (per head: 64 dims + ones col)
  S_T = kpT_h.T-chunks @ qpT_h    [SKV, S] per head (2 heads row-packed, K=64)
  E_T = exp(S_T)                  (no max-subtraction: |energy| <~ 60, safe)
  AV  = vp_aug.T @ E_T  -> [65, S]: rows 0-63 = unnormalized O_T, row 64 =
        softmax denominator (ones-column trick; pad rows contribute 0)
  O_T = AV[0:64] / AV[64]         (recip + PE partition-broadcast)
  out_partial = O_T.T @ Wo_g.T    [S, E] (both head-pairs accumulated in PSUM)

Mask handling is exact: masked K/V rows are removed on the host (gather),
so softmax(where(mask==0, -1e20, e)) == exp(e_valid)/sum(exp(e_valid)).

Pipeline: the attention loop is software-pipelined over 512-wide query
blocks: block s emits [AV of block s-1 | scores of block s | exp of block s]
interleaved per skv-chunk, so the Scalar engine (exp, the per-block
bottleneck) always has scores available and the PE back-fills with AV,
Q-projection (pt=0) or fc_out (pt=1) work. exp runs as one 1024-wide
instruction per skv-chunk over both row-packed heads (2 adjacent PSUM
banks) to amortize ACT fixed overheads. Softmax normalization is per-block
so fc_out can start before the whole pair finishes.
"""

import os

import ml_dtypes
import numpy as np

B, S, E, H = 2, 2048, 1024, 16
D = E // H           # 64
NCORES = 8
GROUPS = 4           # head groups per batch (cores per batch)
HPG = H // GROUPS    # 4 heads per core
DC = E // GROUPS     # 256 dims per core
NB = E // 128        # 8 contraction chunks over E
QB = 512             # query block width
NQB = S // QB        # 4

_CACHE = {}


def _split_excess_waits(nc, max_waits=1):
    """walrus rejects instructions carrying >1 sem wait; spread extras onto
    single-wait NoOps inserted before the instruction on the same engine."""
    import concourse.mybir as mybir

    n_split = 0
    for f in nc.m.functions:
        for bb in f.blocks:
            out, changed = [], False
            for ins in bb.instructions:
                si = ins.sync_info
                if si is not None and si.on_wait is not None and len(si.on_wait) > max_waits:
                    waits = list(si.on_wait)
                    for w in waits[:-max_waits]:
                        out.append(mybir.InstNoOp(
                            name=nc.get_next_instruction_name(),
                            engine=ins.engine, ins=[], outs=[],
                            sync_info=mybir.SyncInfo(on_wait=[w], on_update=[])))
                        n_split += 1
                    ins.sync_info = mybir.SyncInfo(
                        on_wait=waits[-max_waits:], on_update=list(si.on_update))
                    changed = True
                out.append(ins)
            if changed:
                bb.instructions = out
    return n_split


def _build(skv, split_waits=True):
    import concourse.bass as bass
    import concourse.mybir as mybir
    import concourse.tile as tile

    f32 = mybir.dt.float32
    f32r = mybir.dt.float32r
    f16 = mybir.dt.float16
    bf16 = mybir.dt.bfloat16
    f8 = mybir.dt.float8e4
    Alu = mybir.AluOpType
    Act = mybir.ActivationFunctionType
    DR = mybir.MatmulPerfMode.DoubleRow

    nsk = skv // 128
    kblocks = []
    rem = skv
    while rem > 0:
        w = 384 if rem % 384 == 0 else min(256, rem)
        kblocks.append(w)
        rem -= w

    nc = bass.Bass()
    # all streams/weights arrive pre-packed host-side as [128, ...] with
    # long contiguous per-partition runs for DMA efficiency
    xqP = nc.declare_dram_parameter("xqP", [128, NQB * NB * 512], f16, isOutput=False)
    xkP = nc.declare_dram_parameter("xkP", [128, NB * skv], f16, isOutput=False)
    xvP = nc.declare_dram_parameter("xvP", [128, NB * skv], f16, isOutput=False)
    wqP = nc.declare_dram_parameter("wqP", [128, NB * DC], f16, isOutput=False)
    wkP = nc.declare_dram_parameter("wkP", [128, NB * DC], f16, isOutput=False)
    wvP = nc.declare_dram_parameter("wvP", [128, NB * DC], f16, isOutput=False)
    woP = nc.declare_dram_parameter("woP", [128, 2 * E], f16, isOutput=False)
    bq_d = nc.declare_dram_parameter("bq", [DC], f32, isOutput=False)
    bk_d = nc.declare_dram_parameter("bk", [DC], f32, isOutput=False)
    bv_d = nc.declare_dram_parameter("bv", [DC], f32, isOutput=False)
    vm_d = nc.declare_dram_parameter("vmask", [skv], f32, isOutput=False)
    sel2_d = nc.declare_dram_parameter("sel2", [2, 128], f32r, isOutput=False)
    out_d = nc.declare_dram_parameter("out", [S, E], f16, isOutput=True)

    with tile.TileContext(nc) as tc:
        with (
            tc.tile_pool(name="weights", bufs=1) as wpool,
            tc.tile_pool(name="consts", bufs=1) as cpool,
            tc.tile_pool(name="persist", bufs=1) as ppool,
            tc.tile_pool(name="small", bufs=2) as smpool,
            tc.tile_pool(name="stream", bufs=1) as spool,
            tc.tile_pool(name="px_ps", bufs=2, space="PSUM") as pxps,
            tc.tile_pool(name="av_ps", bufs=2, space="PSUM") as avps,
            tc.tile_pool(name="gen_ps", bufs=2, space="PSUM") as gps,
            tc.tile_pool(name="et", bufs=2) as etpool,
            tc.tile_pool(name="outp", bufs=3) as opool,
            tc.tile_pool(name="rcr", bufs=2) as rcpool,
        ):
            # ---- tiles ----
            wk_t = wpool.tile([128, NB, DC], f16, tag="wk", name="wk_t")
            wq_t = wpool.tile([128, NB, DC], f16, tag="wq", name="wq_t")
            wv_t = wpool.tile([128, NB, DC], f16, tag="wv", name="wv_t")
            wo_t = wpool.tile([128, DC // 128, E], f16, tag="wo", name="wo_t")
            bq_t = cpool.tile([128, 2], f32, tag="bq")
            bk_t = cpool.tile([128, 2], f32, tag="bk")
            bv_t = cpool.tile([128, DC], f32, tag="bv")
            vm_t = cpool.tile([128, nsk], f32, tag="vm")
            sel2_t = cpool.tile([2, 128], f32r, tag="sel2")
            xks = [spool.tile([128, NB, w], f16, tag=f"xk{i}", name="xk")
                   for i, w in enumerate(kblocks)]
            xqs = [spool.tile([128, NB, 512], f16, tag=f"xq{i}", name="xq")
                   for i in range(NQB)]
            xvs = [spool.tile([128, NB, 128], f16, tag=f"xv{i}", name="xv")
                  for i in range(nsk)]

            qpT = ppool.tile([128, 2, S], f16, tag="qpT")
            kpT = ppool.tile([128, 2, skv], f16, tag="kpT")
            vp = ppool.tile([128, nsk, HPG * (D + 1)], bf16, tag="vp")
            # per-j unnormalized O_T (rows 0-63) + denominator (row 64)
            o_un0 = ppool.tile([65, 2, S], f32, tag="o_un0")
            o_un1 = ppool.tile([65, 2, S], f32, tag="o_un1")
            o_uns = [o_un0, o_un1]
            o_f16 = ppool.tile([128, 2, S], f16, tag="o_f16")

            # ---- all input DMAs issued upfront in priority order; the
            # late-needed ones go on the scalar HWDGE queue (idle pre-exp) ----
            nc.sync.dma_start(sel2_t[:], sel2_d[:])
            nc.sync.dma_start(wk_t[:], wkP[:])
            nc.sync.dma_start(bk_t[:], bk_d.rearrange("(c p) -> p c", p=128))
            nc.sync.dma_start(bq_t[:], bq_d.rearrange("(c p) -> p c", p=128))
            nc.sync.dma_start(bv_t[:], bv_d[None, :].to_broadcast((128, DC)))
            nc.sync.dma_start(vm_t[:], vm_d.rearrange("(s p) -> p s", p=128))
            koff = [0]
            for w in kblocks:
                koff.append(koff[-1] + NB * w)
            nc.sync.dma_start(xks[0][:], xkP[:, koff[0]:koff[1]])
            nc.sync.dma_start(wq_t[:], wqP[:])
            nc.sync.dma_start(xqs[0][:], xqP[:, 0:NB * 512])
            for i in range(1, len(kblocks)):
                nc.sync.dma_start(xks[i][:], xkP[:, koff[i]:koff[i + 1]])
            nc.sync.dma_start(wv_t[:], wvP[:])
            for sc in range(nsk):
                nc.sync.dma_start(
                    xvs[sc][:], xvP[:, sc * NB * 128:(sc + 1) * NB * 128])
            for nb in range(1, NQB):
                nc.sync.dma_start(
                    xqs[nb][:], xqP[:, nb * NB * 512:(nb + 1) * NB * 512])
            nc.sync.dma_start(wo_t[:], woP[:])


            def mm_k():
                off = 0
                for i, w in enumerate(kblocks):
                    for mc in range(2):
                        ps = gps.tile([128, 512], f32, tag="gp", name="kp_ps")[:, :w]
                        for kc in range(NB):
                            nc.tensor.matmul(
                                ps[:], wk_t[:, kc, mc * 128:(mc + 1) * 128],
                                xks[i][:, kc, :], start=(kc == 0), stop=(kc == NB - 1))
                        nc.vector.tensor_tensor(
                            out=kpT[:, mc, off:off + w], in0=ps[:],
                            in1=bk_t[:, mc:mc + 1].to_broadcast((128, w)), op=Alu.add)
                    off += w

            def mm_q(nb, mc):
                ps = gps.tile([128, 512], f32, tag="gp", name="qp_ps")
                for kc in range(NB):
                    nc.tensor.matmul(
                        ps[:], wq_t[:, kc, mc * 128:(mc + 1) * 128],
                        xqs[nb][:, kc, :], start=(kc == 0), stop=(kc == NB - 1))
                nc.vector.tensor_tensor(
                    out=qpT[:, mc, nb * 512:(nb + 1) * 512], in0=ps[:],
                    in1=bq_t[:, mc:mc + 1].to_broadcast((128, 512)), op=Alu.add)

            def mm_v(sc):
                ps = gps.tile([128, 512], f32, tag="gp", name="vp_ps")[:, :DC]
                for kc in range(NB):
                    nc.tensor.matmul(
                        ps[:], xvs[sc][:, kc, :], wv_t[:, kc, :],
                        start=(kc == 0), stop=(kc == NB - 1))
                t1 = smpool.tile([128, DC], f32, tag="vtmp")
                nc.vector.tensor_tensor(out=t1[:], in0=ps[:], in1=bv_t[:], op=Alu.add)
                vps = vp[:, sc, :].rearrange("p (h w) -> p h w", w=D + 1)
                nc.vector.tensor_tensor(
                    out=vps[:, :, 0:D],
                    in0=t1.rearrange("p (h w) -> p h w", w=D),
                    in1=vm_t[:, sc:sc + 1, None].to_broadcast((128, HPG, D)),
                    op=Alu.mult)
                nc.vector.tensor_copy(
                    out=vps[:, :, D:D + 1],
                    in_=vm_t[:, sc:sc + 1, None].to_broadcast((128, HPG, 1)))

            def finish_block(pt, qb, psavs):
                """AV copy + per-block softmax normalize for query block qb
                of pair pt (runs one pipeline step after its AV matmuls).
                psavs=None when the caller already copied AV to o_un."""
                q0 = qb * QB
                if psavs is not None:
                    for j in range(2):
                        nc.vector.tensor_copy(
                            out=o_uns[j][0:D + 1, pt, q0:q0 + QB],
                            in_=psavs[j][0:D + 1, :])
                rc_r = rcpool.tile([2, QB], f32r, tag="rcr")
                for j in range(2):
                    s128 = smpool.tile([128, QB // 128], f32, tag="s128")
                    nc.gpsimd.dma_start(s128[:], o_uns[j][D:D + 1, pt, q0:q0 + QB])
                    nc.vector.reciprocal(out=s128[:], in_=s128[:])
                    r128 = smpool.tile([128, QB // 128], f32r, tag="r128")
                    nc.vector.tensor_copy(out=r128[:], in_=s128[:])
                    nc.gpsimd.dma_start(rc_r[j:j + 1, :], r128[:])
                # one PE broadcast for both heads: sel2 routes row j of rc_r
                # to partitions 64j..64j+63
                rc_ps = gps.tile([128, 512], f32, tag="gp", name="rc_ps")
                nc.tensor.matmul(
                    rc_ps[:], sel2_t[:], rc_r[:], start=True, stop=True)
                for j in range(2):
                    nc.vector.tensor_tensor(
                        out=o_f16[64 * j:64 * j + 64, pt, q0:q0 + QB],
                        in0=o_uns[j][0:D, pt, q0:q0 + QB],
                        in1=rc_ps[64 * j:64 * j + 64, :], op=Alu.mult)

            def fc_block(b):
                """fc_out for query block b (both pairs accumulated in PSUM)."""
                for sqc in range(b * (QB // 128), (b + 1) * (QB // 128)):
                    ob = opool.tile([128, 2, 512], f16, tag="ob")
                    for eb in range(2):
                        ps = gps.tile([128, 512], f32, tag="gp", name="fc_ps")
                        nc.tensor.matmul(
                            ps[:], o_f16[:, 0, sqc * 128:(sqc + 1) * 128],
                            wo_t[:, 0, eb * 512:(eb + 1) * 512],
                            start=True, stop=False)
                        nc.tensor.matmul(
                            ps[:], o_f16[:, 1, sqc * 128:(sqc + 1) * 128],
                            wo_t[:, 1, eb * 512:(eb + 1) * 512],
                            start=False, stop=True)
                        nc.vector.tensor_copy(out=ob[:, eb, :], in_=ps[:])
                    nc.sync.dma_start(
                        out_d[sqc * 128:(sqc + 1) * 128, :], ob[:])

            # HAM warm-up: junk matmuls at idle-filler priority keep the PE
            # activity window busy while the input streams arrive, so the
            # real projection matmuls run at full clock. Emitted before mm_k
            # so their PSUM ring slots precede the projections'.
            with tc.high_priority(offset=-10**6):
                for _ in range(12):
                    wps = gps.tile([128, 512], f32, tag="gp", name="warm_ps")
                    nc.tensor.matmul(wps[:, :128], sel2_t[:], sel2_t[:],
                                     start=True, stop=True)
            # lead-in: kpT fully, first half of qpT block 0 (pair 0)
            mm_k()
            mm_q(0, 0)

            def av_chunk(state, psavs, sk0, sk1):
                """AV matmuls for skc in [sk0, sk1) of the previous block,
                one contiguous accumulation chain per head."""
                ppt, pqb, pet = state
                for j in range(2):
                    hl = 2 * ppt + j
                    for skc in range(sk0, sk1):
                        nc.tensor.matmul(
                            psavs[j][0:D + 1, :],
                            vp[:, skc, hl * (D + 1):(hl + 1) * (D + 1)],
                            pet[:, skc, j, :],
                            start=(skc == 0), stop=(skc == nsk - 1))

            # software-pipelined attention over 8 (pt, qb) steps
            state = None  # (pt, qb, et tile) awaiting AV
            GRP = 4
            for pt in range(2):
                for qb in range(NQB):
                    q0 = qb * QB
                    et_t = etpool.tile([128, nsk, 2, QB], bf16, tag="et",
                                       name="et_t")
                    psavs = None
                    if state is not None:
                        psavs = [avps.tile([128, QB], f32, tag="av",
                                           name=f"psav{j}") for j in range(2)]
                    for g0 in range(0, nsk, GRP):
                        g1 = min(g0 + GRP, nsk)
                        if state is not None:
                            av_chunk(state, psavs, g0, g1)
                        for skc in range(g0, g1):
                            psx = pxps.tile([128, 2, QB], f32, tag="px",
                                            name="psx")
                            for j in range(2):
                                nc.tensor.matmul(
                                    psx[:, j, :],
                                    kpT[64 * j:64 * j + 64, pt,
                                        skc * 128:(skc + 1) * 128],
                                    qpT[64 * j:64 * j + 64, pt, q0:q0 + QB],
                                    start=True, stop=True,
                                    tile_position=(64 * j, 0))
                            nc.scalar.activation(
                                et_t[:, skc, :, :], psx[:], Act.Exp)
                    # PE filler during the ACT-bound exp phase. In pt=0 the
                    # filler is emitted before finish_block so its matmuls
                    # outrank the normalize chain; in pt=1 fc needs the
                    # normalize, so finish goes first. Each step carries one
                    # or two q-projection halves (pair-0 halves just in time
                    # for pt=0, pair-1 halves ahead of pt=1).
                    if pt == 0:
                        if qb + 1 < NQB:
                            mm_q(qb + 1, 0)
                        if qb >= 1:
                            mm_q(qb - 1, 1)
                        if qb == 0:
                            # vp must be complete before block-0's AV, which
                            # is emitted at the start of step 1
                            for sc in range(nsk):
                                mm_v(sc)
                        if state is not None:
                            finish_block(state[0], state[1], psavs)
                    else:
                        if qb == 0:
                            mm_q(NQB - 1, 1)
                        if state is not None:
                            finish_block(state[0], state[1], psavs)
                        if qb >= 1:
                            fc_block(qb - 1)
                    state = (pt, qb, et_t)
            # drain: AV + normalize of the last block, then remaining fc.
            # j0's AV chain and copy go first so its reciprocal round-trip
            # overlaps j1's AV matmuls.
            psavs = [avps.tile([128, QB], f32, tag="av", name=f"psavd{j}")
                     for j in range(2)]
            ppt, pqb, pet = state
            q0 = pqb * QB
            for j in range(2):
                hl = 2 * ppt + j
                for skc in range(nsk):
                    nc.tensor.matmul(
                        psavs[j][0:D + 1, :],
                        vp[:, skc, hl * (D + 1):(hl + 1) * (D + 1)],
                        pet[:, skc, j, :],
                        start=(skc == 0), stop=(skc == nsk - 1))
                nc.vector.tensor_copy(
                    out=o_uns[j][0:D + 1, ppt, q0:q0 + QB],
                    in_=psavs[j][0:D + 1, :])
            finish_block(ppt, pqb, None)
            fc_block(3)

    if split_waits:
        _split_excess_waits(nc)
    return nc


def _pack_stream(xT, widths):
    """[E, total] -> [128, sum(NB*w)]: per width-block, layout
    [p, ko, s] flattened so each partition's run is contiguous."""
    r = xT.reshape(NB, 128, xT.shape[1])
    parts, off = [], 0
    for w in widths:
        parts.append(np.ascontiguousarray(
            r[:, :, off:off + w].transpose(1, 0, 2)).reshape(128, NB * w))
        off += w
    return np.concatenate(parts, axis=1)


def _pack_w(wT):
    """[E, M] -> [128, NB*M] in [p, ko, m] layout."""
    M = wT.shape[1]
    return np.ascontiguousarray(
        wT.reshape(NB, 128, M).transpose(1, 0, 2)).reshape(128, NB * M)


def _prep_inputs(q, k, v, mask, W_qkv, b_qkv, W_out, b_out):
    """Host-side shard/layout prep. Returns (skv, in_maps)."""
    q = np.asarray(q, dtype=np.float32)
    k = np.asarray(k, dtype=np.float32)
    v = np.asarray(v, dtype=np.float32)
    mask = np.asarray(mask)
    W_qkv = np.asarray(W_qkv, dtype=np.float32)
    b_qkv = np.asarray(b_qkv, dtype=np.float32)
    W_out = np.asarray(W_out, dtype=np.float32)

    valid = [np.nonzero(mask[b, 0, 0] != 0)[0] for b in range(B)]
    cnts = [len(vi) for vi in valid]
    skv = max(128, max((c + 127) // 128 * 128 for c in cnts))
    kblocks = []
    rem = skv
    while rem > 0:
        w = 384 if rem % 384 == 0 else min(256, rem)
        kblocks.append(w)
        rem -= w

    # per-batch packed streams
    xqPs, xkPs, xvPs, vms = [], [], [], []
    for b in range(B):
        qT = q[b].T.astype(np.float16)
        kt = np.zeros((E, skv), np.float16)
        vt = np.zeros((E, skv), np.float16)
        kt[:, :cnts[b]] = k[b][valid[b]].T
        vt[:, :cnts[b]] = v[b][valid[b]].T
        xqPs.append(_pack_stream(qT, [512] * NQB))
        xkPs.append(_pack_stream(kt, kblocks))
        xvPs.append(_pack_stream(vt, [128] * (skv // 128)))
        vm = np.zeros((skv,), np.float32)
        vm[:cnts[b]] = 1.0
        vms.append(vm)

    sel2 = np.zeros((2, 128), np.float32)
    sel2[0, :64] = 1.0
    sel2[1, 64:] = 1.0

    in_maps = []
    for c in range(NCORES):
        b, g = divmod(c, GROUPS)
        sl = slice(g * DC, (g + 1) * DC)
        woT = W_out[:, sl].T.astype(np.float16)  # [DC, E]
        woP = np.ascontiguousarray(
            woT.reshape(2, 128, E).transpose(1, 0, 2)).reshape(128, 2 * E)
        in_maps.append({
            "xqP": xqPs[b], "xkP": xkPs[b], "xvP": xvPs[b],
            "wqP": _pack_w(W_qkv[sl, :].T.astype(np.float16)),
            "wkP": _pack_w(W_qkv[E:][sl, :].T.astype(np.float16)),
            "wvP": _pack_w(W_qkv[2 * E:][sl, :].T.astype(np.float16)),
            "woP": woP,
            "bq": np.ascontiguousarray(b_qkv[sl]),
            "bk": np.ascontiguousarray(b_qkv[E:][sl]),
            "bv": np.ascontiguousarray(b_qkv[2 * E:][sl]),
            "vmask": vms[b],
            "sel2": sel2,
        })
    return skv, in_maps


def kernel(q, k, v, mask, W_qkv, b_qkv, W_out, b_out):
    from concourse import bass_utils

    skv, in_maps = _prep_inputs(q, k, v, mask, W_qkv, b_qkv, W_out, b_out)
    if skv not in _CACHE:
        _CACHE[skv] = _build(skv)
    nc = _CACHE[skv]

    trace = os.environ.get("KERNEL_TRACE") == "1"
    if trace:
        bass_utils.upload_artifacts = lambda tmpdir: "local://" + tmpdir
    res = bass_utils.run_bass_kernel_spmd(
        nc, in_maps, list(range(NCORES)), trace=trace)
    if trace:
        print(f"HW exec time: {res.exec_time_ns} ns")

    b_out = np.asarray(b_out, dtype=np.float32)
    out = np.zeros((B, S, E), np.float32)
    for c in range(NCORES):
        out[c // GROUPS] += res.results[c]["out"].astype(np.float32)
    out += b_out[None, None, :]
    return out
